# revision 7
# baseline (speedup 1.0000x reference)
# MoE routing kernel for Trainium2 (Bass/Tile), SPMD over 8 NeuronCores.
#
# Reference computation (B=4, T=2048, D=H=1024, V=8, L=4):
#   h      = gelu(einsum("btd,vdh->btvh", X, W1) + b1)
#   outs   = einsum("btvh,vhk->btvk", h, W2) + b2
#   w      = softmax(op_logits, axis=-1)            # [B, L, V]
#   result = einsum("blv,btvh->bth", w, outs) / L
#
# Strategy:
#   - Host: softmax + mean over L -> wbar[B, V]; fold b2 into a single
#     per-batch combined bias cbias[b] = sum_v wbar[b,v] * b2[v].
#   - Data parallel over tokens: core c owns tokens [c*1024, (c+1)*1024).
#     Each 1024-token shard lies inside a single batch row b, so wbar/cbias
#     are per-core constants (shipped as data => one SPMD program).
#   - Per core, per expert v:
#       MM1:  pre1^T[h, t] = sum_d W1[v][d, h]^T-free X^T[d, t]   (PE)
#       gelu: h_sb[h, t] = Gelu(pre1 + b1[v][h])                  (ACT, PSUM->SBUF)
#       MM2:  out[t, k]  = sum_h h_sb[h, t]-as-lhsT W2[v][h, k]   (PE)
#       acc:  out_acc[t, k] = wbar[v] * out + (cbias | out_acc)   (DVE)
#   - X is pre-transposed on host to [D, BT] so every matmul operand is
#     naturally contraction-major; no on-device transposes anywhere.

import os

import numpy as np
import ml_dtypes

import concourse.bass as bass
import concourse.mybir as mybir
import concourse.tile as tile
from concourse import bacc
from concourse.bass_utils import run_bass_kernel_spmd

N_CORES = 8
P = 128

_DT_MAP = {
    "bf16": mybir.dt.bfloat16,
    "f32r": mybir.dt.float32r,
    "f32": mybir.dt.float32,
}
_NP_DT_MAP = {
    "bf16": ml_dtypes.bfloat16,
    "f32r": np.float32,
    "f32": np.float32,
}


def build_moe_core_program(TC, D, H, V, mode="bf16", act="gelu"):
    """One NeuronCore's program: TC tokens, full V experts."""
    act_func = {
        "gelu": mybir.ActivationFunctionType.Gelu,
        "tanh": mybir.ActivationFunctionType.Tanh,  # sim-only (CoreSim lacks Gelu)
    }[act]
    DT = _DT_MAP[mode]
    f32 = mybir.dt.float32
    DC = D // P          # contraction chunks for MM1
    HC = H // P          # contraction chunks for MM2
    NT = min(512, TC)    # MM1 moving free dim (tokens)
    NK = min(512, H)     # MM2 moving free dim (output cols)
    TT = TC // P         # token tiles of 128

    # Bacc (not plain Bass): its finalize() runs generate_event_semaphores,
    # which splits multi-sem waits — TRN2 allows max 1 wait per instruction.
    nc = bacc.Bacc(trn_type="TRN2")
    x_t = nc.declare_dram_parameter("x_t", [D, TC], DT, isOutput=False)
    w1 = nc.declare_dram_parameter("w1", [V, D, H], DT, isOutput=False)
    w2 = nc.declare_dram_parameter("w2", [V, H, H], DT, isOutput=False)
    b1t = nc.declare_dram_parameter("b1t", [H, V], f32, isOutput=False)
    wbar = nc.declare_dram_parameter("wbar", [P, V], f32, isOutput=False)
    cbias = nc.declare_dram_parameter("cbias", [P, H], f32, isOutput=False)
    out = nc.declare_dram_parameter("out", [TC, H], f32, isOutput=True)

    with tile.TileContext(nc) as tc:
        with (
            tc.tile_pool(name="const", bufs=1) as cpool,
            tc.tile_pool(name="w1p", bufs=2) as w1p,
            tc.tile_pool(name="w2p", bufs=2) as w2p,
            tc.tile_pool(name="hbuf", bufs=1) as hpool,
            tc.tile_pool(name="accp", bufs=1) as accp,
            tc.tile_pool(name="ps1", bufs=4, space="PSUM") as ps1,
            tc.tile_pool(name="ps2", bufs=4, space="PSUM") as ps2,
        ):
            x_sb = cpool.tile([P, DC, TC], DT)
            nc.sync.dma_start(out=x_sb, in_=x_t.rearrange("(dc p) t -> p dc t", p=P))
            b1_sb = cpool.tile([P, HC, V], f32)
            nc.sync.dma_start(out=b1_sb, in_=b1t.rearrange("(hc p) v -> p hc v", p=P))
            wbar_sb = cpool.tile([P, V], f32)
            nc.sync.dma_start(out=wbar_sb, in_=wbar[:])
            cbias_sb = cpool.tile([P, H], f32)
            nc.sync.dma_start(out=cbias_sb, in_=cbias[:])
            out_acc = accp.tile([P, TT, H], f32)

            for v in range(V):
                w1_sb = w1p.tile([P, DC, H], DT, tag="w1")
                nc.sync.dma_start(
                    out=w1_sb, in_=w1[v].rearrange("(dc p) h -> p dc h", p=P)
                )
                w2_sb = w2p.tile([P, HC, H], DT, tag="w2")
                nc.sync.dma_start(
                    out=w2_sb, in_=w2[v].rearrange("(hc p) k -> p hc k", p=P)
                )
                h_sb = hpool.tile([P, HC, TC], DT, tag="h")

                # MM1 + gelu: produce h-major activations h_sb[h, t]
                for hc in range(HC):
                    for th in range(TC // NT):
                        p1 = ps1.tile([P, NT], f32, tag="p1")
                        for dc in range(DC):
                            nc.tensor.matmul(
                                p1,
                                w1_sb[:, dc, hc * P:(hc + 1) * P],
                                x_sb[:, dc, th * NT:(th + 1) * NT],
                                start=(dc == 0),
                                stop=(dc == DC - 1),
                            )
                        nc.scalar.activation(
                            h_sb[:, hc, th * NT:(th + 1) * NT],
                            p1,
                            act_func,
                            bias=b1_sb[:, hc, v:v + 1],
                        )

                # MM2 + weighted accumulate into out_acc[t, k]
                for tt in range(TT):
                    for kc in range(H // NK):
                        p2 = ps2.tile([P, NK], f32, tag="p2")
                        for hc in range(HC):
                            nc.tensor.matmul(
                                p2,
                                h_sb[:, hc, tt * P:(tt + 1) * P],
                                w2_sb[:, hc, kc * NK:(kc + 1) * NK],
                                start=(hc == 0),
                                stop=(hc == HC - 1),
                            )
                        ksl = slice(kc * NK, (kc + 1) * NK)
                        in1 = cbias_sb[:, ksl] if v == 0 else out_acc[:, tt, ksl]
                        nc.vector.scalar_tensor_tensor(
                            out=out_acc[:, tt, ksl],
                            in0=p2,
                            scalar=wbar_sb[:, v:v + 1],
                            in1=in1,
                            op0=mybir.AluOpType.mult,
                            op1=mybir.AluOpType.add,
                        )

            nc.sync.dma_start(
                out=out.rearrange("(tt p) k -> p tt k", p=P), in_=out_acc
            )
    nc.finalize()  # Bacc: runs wait-splitting + reg alloc passes
    return nc


_prog_cache = {}


def _get_program(mode, TC, D, H, V):
    key = (mode, TC, D, H, V)
    if key not in _prog_cache:
        _prog_cache[key] = build_moe_core_program(TC, D, H, V, mode=mode)
    return _prog_cache[key]


def host_prep(op_logits, token_feats, W1, b1, W2, b2, mode):
    """Shared host-side preprocessing: softmax folding, transpose, cast, shard."""
    op_logits = np.asarray(op_logits, dtype=np.float32)
    token_feats = np.asarray(token_feats, dtype=np.float32)
    W1 = np.asarray(W1, dtype=np.float32)
    b1 = np.asarray(b1, dtype=np.float32)
    W2 = np.asarray(W2, dtype=np.float32)
    b2 = np.asarray(b2, dtype=np.float32)

    B, T, D = token_feats.shape
    V, _, H = W1.shape
    BT = B * T
    TC = BT // N_CORES

    lg = op_logits.astype(np.float64)
    e = np.exp(lg - lg.max(axis=-1, keepdims=True))
    w = e / e.sum(axis=-1, keepdims=True)
    wbar = w.mean(axis=1)                       # [B, V], includes the 1/L
    cbias = wbar @ b2.astype(np.float64)        # [B, H]

    np_dt = _NP_DT_MAP[mode]
    x_t = np.ascontiguousarray(token_feats.reshape(BT, D).T).astype(np_dt)
    w1c = np.ascontiguousarray(W1.astype(np_dt))
    w2c = np.ascontiguousarray(W2.astype(np_dt))
    b1t = np.ascontiguousarray(b1.T.astype(np.float32))

    in_maps = []
    for c in range(N_CORES):
        bc = (c * TC) // T
        in_maps.append({
            "x_t": np.ascontiguousarray(x_t[:, c * TC:(c + 1) * TC]),
            "w1": w1c,
            "w2": w2c,
            "b1t": b1t,
            "wbar": np.ascontiguousarray(
                np.broadcast_to(wbar[bc].astype(np.float32), (P, V))
            ),
            "cbias": np.ascontiguousarray(
                np.broadcast_to(cbias[bc].astype(np.float32), (P, H))
            ),
        })
    return in_maps, (B, T, D, H, V, TC)


LAST_RESULTS = None


def kernel(op_logits, token_feats, W1, b1, W2, b2):
    global LAST_RESULTS
    mode = os.environ.get("MOE_DTYPE", "bf16")
    in_maps, (B, T, D, H, V, TC) = host_prep(
        op_logits, token_feats, W1, b1, W2, b2, mode
    )
    nc = _get_program(mode, TC, D, H, V)
    res = run_bass_kernel_spmd(
        nc,
        in_maps,
        list(range(N_CORES)),
        trace=os.environ.get("MOE_TRACE", "0") == "1",
    )
    LAST_RESULTS = res
    outs = [res.results[c]["out"] for c in range(N_CORES)]
    return np.concatenate(outs, axis=0).reshape(B, T, H).astype(np.float32)


# revision 9
# speedup vs baseline: 1.0106x; 1.0106x over previous
# MoE routing kernel for Trainium2 (Bass/Tile), SPMD over 8 NeuronCores.
#
# Reference computation (B=4, T=2048, D=H=1024, V=8, L=4):
#   h      = gelu(einsum("btd,vdh->btvh", X, W1) + b1)
#   outs   = einsum("btvh,vhk->btvk", h, W2) + b2
#   w      = softmax(op_logits, axis=-1)            # [B, L, V]
#   result = einsum("blv,btvh->bth", w, outs) / L
#
# Strategy:
#   - Host: softmax + mean over L -> wbar[B, V]; fold b2 into a single
#     per-batch combined bias cbias[b] = sum_v wbar[b,v] * b2[v].
#   - Data parallel over tokens: core c owns tokens [c*1024, (c+1)*1024).
#     Each 1024-token shard lies inside a single batch row b, so wbar/cbias
#     are per-core constants (shipped as data => one SPMD program).
#   - Per core, per expert v:
#       MM1:  pre1^T[h, t] = sum_d W1[v][d, h]^T-free X^T[d, t]   (PE)
#       gelu: h_sb[h, t] = Gelu(pre1 + b1[v][h])                  (ACT, PSUM->SBUF)
#       MM2:  out[t, k]  = sum_h h_sb[h, t]-as-lhsT W2[v][h, k]   (PE)
#       acc:  out_acc[t, k] = wbar[v] * out + (cbias | out_acc)   (DVE)
#   - X is pre-transposed on host to [D, BT] so every matmul operand is
#     naturally contraction-major; no on-device transposes anywhere.

import os

import numpy as np
import ml_dtypes

import concourse.bass as bass
import concourse.mybir as mybir
import concourse.tile as tile
from concourse import bacc
from concourse.bass_utils import run_bass_kernel_spmd

N_CORES = 8
P = 128

_DT_MAP = {
    "bf16": mybir.dt.bfloat16,
    "f32r": mybir.dt.float32r,
    "f32": mybir.dt.float32,
}
_NP_DT_MAP = {
    "bf16": ml_dtypes.bfloat16,
    "f32r": np.float32,
    "f32": np.float32,
}


def build_moe_core_program(TC, D, H, V, mode="bf16", act="gelu"):
    """One NeuronCore's program: TC tokens, full V experts."""
    act_func = {
        "gelu": mybir.ActivationFunctionType.Gelu,
        "tanh": mybir.ActivationFunctionType.Tanh,  # sim-only (CoreSim lacks Gelu)
    }[act]
    DT = _DT_MAP[mode]
    f32 = mybir.dt.float32
    DC = D // P          # contraction chunks for MM1
    HC = H // P          # contraction chunks for MM2
    NT = min(512, TC)    # MM1 moving free dim (tokens)
    NK = min(512, H)     # MM2 moving free dim (output cols)
    TT = TC // P         # token tiles of 128

    # Bacc (not plain Bass): its finalize() runs generate_event_semaphores,
    # which splits multi-sem waits — TRN2 allows max 1 wait per instruction.
    nc = bacc.Bacc(trn_type="TRN2")
    x_t = nc.declare_dram_parameter("x_t", [D, TC], DT, isOutput=False)
    w1 = nc.declare_dram_parameter("w1", [V, D, H], DT, isOutput=False)
    w2 = nc.declare_dram_parameter("w2", [V, H, H], DT, isOutput=False)
    b1t = nc.declare_dram_parameter("b1t", [H, V], f32, isOutput=False)
    wbar = nc.declare_dram_parameter("wbar", [P, V], f32, isOutput=False)
    cbias = nc.declare_dram_parameter("cbias", [P, H], f32, isOutput=False)
    out = nc.declare_dram_parameter("out", [TC, H], f32, isOutput=True)

    with tile.TileContext(nc) as tc:
        with (
            tc.tile_pool(name="const", bufs=1) as cpool,
            tc.tile_pool(name="w1p", bufs=2) as w1p,
            tc.tile_pool(name="w2p", bufs=2) as w2p,
            tc.tile_pool(name="hbuf", bufs=1) as hpool,
            tc.tile_pool(name="accp", bufs=1) as accp,
            tc.tile_pool(name="ps1", bufs=4, space="PSUM") as ps1,
            tc.tile_pool(name="ps2", bufs=4, space="PSUM") as ps2,
        ):
            NTH = TC // NT  # token halves
            # x split per (dc, th): the first matmul group's deps are only the
            # slices it reads, so PE starts as soon as those chunks land.
            x_tiles = {}
            for dc in range(DC):
                for th in range(NTH):
                    xt = cpool.tile([P, NT], DT, tag=f"x{dc}_{th}")
                    nc.sync.dma_start(
                        out=xt,
                        in_=x_t[dc * P:(dc + 1) * P, th * NT:(th + 1) * NT],
                    )
                    x_tiles[(dc, th)] = xt
            b1_sb = cpool.tile([P, HC, V], f32)
            nc.sync.dma_start(out=b1_sb, in_=b1t.rearrange("(hc p) v -> p hc v", p=P))
            wbar_sb = cpool.tile([P, V], f32)
            nc.sync.dma_start(out=wbar_sb, in_=wbar[:])
            cbias_sb = cpool.tile([P, H], f32)
            nc.sync.dma_start(out=cbias_sb, in_=cbias[:])
            # per-tt output accumulators: each tt's store depends only on its
            # own tile, so final DMAs overlap the last expert's compute.
            out_tiles = [
                accp.tile([P, H], f32, tag=f"acc{tt}", name=f"acc{tt}")
                for tt in range(TT)
            ]
            out_r = out.rearrange("(tt p) k -> p tt k", p=P)

            for v in range(V):
                w1_sb = w1p.tile([P, DC, H], DT, tag="w1")
                nc.sync.dma_start(
                    out=w1_sb, in_=w1[v].rearrange("(dc p) h -> p dc h", p=P)
                )
                h_sb = hpool.tile([P, HC, TC], DT, tag="h")

                # MM1 + gelu: produce h-major activations h_sb[h, t]
                for hc in range(HC):
                    for th in range(NTH):
                        p1 = ps1.tile([P, NT], f32, tag="p1")
                        for dc in range(DC):
                            nc.tensor.matmul(
                                p1,
                                w1_sb[:, dc, hc * P:(hc + 1) * P],
                                x_tiles[(dc, th)][:],
                                start=(dc == 0),
                                stop=(dc == DC - 1),
                            )
                        nc.scalar.activation(
                            h_sb[:, hc, th * NT:(th + 1) * NT],
                            p1,
                            act_func,
                            bias=b1_sb[:, hc, v:v + 1],
                        )

                # w2 emitted after MM1 so its DMA is deprioritized vs x/w1 at
                # kernel start (it's only needed once MM2 begins).
                w2_sb = w2p.tile([P, HC, H], DT, tag="w2")
                nc.sync.dma_start(
                    out=w2_sb, in_=w2[v].rearrange("(hc p) k -> p hc k", p=P)
                )

                # MM2 + weighted accumulate into out_tiles[tt][t, k]
                for tt in range(TT):
                    for kc in range(H // NK):
                        p2 = ps2.tile([P, NK], f32, tag="p2")
                        for hc in range(HC):
                            nc.tensor.matmul(
                                p2,
                                h_sb[:, hc, tt * P:(tt + 1) * P],
                                w2_sb[:, hc, kc * NK:(kc + 1) * NK],
                                start=(hc == 0),
                                stop=(hc == HC - 1),
                            )
                        ksl = slice(kc * NK, (kc + 1) * NK)
                        in1 = cbias_sb[:, ksl] if v == 0 else out_tiles[tt][:, ksl]
                        nc.vector.scalar_tensor_tensor(
                            out=out_tiles[tt][:, ksl],
                            in0=p2,
                            scalar=wbar_sb[:, v:v + 1],
                            in1=in1,
                            op0=mybir.AluOpType.mult,
                            op1=mybir.AluOpType.add,
                        )
                    if v == V - 1:
                        # store this tt as soon as its accumulation finishes
                        nc.sync.dma_start(out=out_r[:, tt, :], in_=out_tiles[tt][:])
    nc.finalize()  # Bacc: runs wait-splitting + reg alloc passes
    return nc


_prog_cache = {}


def _get_program(mode, TC, D, H, V):
    key = (mode, TC, D, H, V)
    if key not in _prog_cache:
        _prog_cache[key] = build_moe_core_program(TC, D, H, V, mode=mode)
    return _prog_cache[key]


def host_prep(op_logits, token_feats, W1, b1, W2, b2, mode):
    """Shared host-side preprocessing: softmax folding, transpose, cast, shard."""
    op_logits = np.asarray(op_logits, dtype=np.float32)
    token_feats = np.asarray(token_feats, dtype=np.float32)
    W1 = np.asarray(W1, dtype=np.float32)
    b1 = np.asarray(b1, dtype=np.float32)
    W2 = np.asarray(W2, dtype=np.float32)
    b2 = np.asarray(b2, dtype=np.float32)

    B, T, D = token_feats.shape
    V, _, H = W1.shape
    BT = B * T
    TC = BT // N_CORES

    lg = op_logits.astype(np.float64)
    e = np.exp(lg - lg.max(axis=-1, keepdims=True))
    w = e / e.sum(axis=-1, keepdims=True)
    wbar = w.mean(axis=1)                       # [B, V], includes the 1/L
    cbias = wbar @ b2.astype(np.float64)        # [B, H]

    np_dt = _NP_DT_MAP[mode]
    x_t = np.ascontiguousarray(token_feats.reshape(BT, D).T).astype(np_dt)
    w1c = np.ascontiguousarray(W1.astype(np_dt))
    w2c = np.ascontiguousarray(W2.astype(np_dt))
    b1t = np.ascontiguousarray(b1.T.astype(np.float32))

    in_maps = []
    for c in range(N_CORES):
        bc = (c * TC) // T
        in_maps.append({
            "x_t": np.ascontiguousarray(x_t[:, c * TC:(c + 1) * TC]),
            "w1": w1c,
            "w2": w2c,
            "b1t": b1t,
            "wbar": np.ascontiguousarray(
                np.broadcast_to(wbar[bc].astype(np.float32), (P, V))
            ),
            "cbias": np.ascontiguousarray(
                np.broadcast_to(cbias[bc].astype(np.float32), (P, H))
            ),
        })
    return in_maps, (B, T, D, H, V, TC)


LAST_RESULTS = None


def kernel(op_logits, token_feats, W1, b1, W2, b2):
    global LAST_RESULTS
    mode = os.environ.get("MOE_DTYPE", "bf16")
    in_maps, (B, T, D, H, V, TC) = host_prep(
        op_logits, token_feats, W1, b1, W2, b2, mode
    )
    nc = _get_program(mode, TC, D, H, V)
    res = run_bass_kernel_spmd(
        nc,
        in_maps,
        list(range(N_CORES)),
        trace=os.environ.get("MOE_TRACE", "0") == "1",
    )
    LAST_RESULTS = res
    outs = [res.results[c]["out"] for c in range(N_CORES)]
    return np.concatenate(outs, axis=0).reshape(B, T, H).astype(np.float32)


# revision 13
# speedup vs baseline: 1.0126x; 1.0020x over previous
# MoE routing kernel for Trainium2 (Bass/Tile), SPMD over 8 NeuronCores.
#
# Reference computation (B=4, T=2048, D=H=1024, V=8, L=4):
#   h      = gelu(einsum("btd,vdh->btvh", X, W1) + b1)
#   outs   = einsum("btvh,vhk->btvk", h, W2) + b2
#   w      = softmax(op_logits, axis=-1)            # [B, L, V]
#   result = einsum("blv,btvh->bth", w, outs) / L
#
# Strategy:
#   - Host: softmax + mean over L -> wbar[B, V]; fold b2 into a single
#     per-batch combined bias cbias[b] = sum_v wbar[b,v] * b2[v].
#   - Data parallel over tokens: core c owns tokens [c*1024, (c+1)*1024).
#     Each 1024-token shard lies inside a single batch row b, so wbar/cbias
#     are per-core constants (shipped as data => one SPMD program).
#   - Per core, per expert v:
#       MM1:  pre1^T[h, t] = sum_d W1[v][d, h]^T-free X^T[d, t]   (PE)
#       gelu: h_sb[h, t] = Gelu(pre1 + b1[v][h])                  (ACT, PSUM->SBUF)
#       MM2:  out[t, k]  = sum_h h_sb[h, t]-as-lhsT W2[v][h, k]   (PE)
#       acc:  out_acc[t, k] = wbar[v] * out + (cbias | out_acc)   (DVE)
#   - X is pre-transposed on host to [D, BT] so every matmul operand is
#     naturally contraction-major; no on-device transposes anywhere.

import os

import numpy as np
import ml_dtypes

import concourse.bass as bass
import concourse.mybir as mybir
import concourse.tile as tile
from concourse import bacc
from concourse.bass_utils import run_bass_kernel_spmd

N_CORES = 8
P = 128

_DT_MAP = {
    "bf16": mybir.dt.bfloat16,
    "f32r": mybir.dt.float32r,
    "f32": mybir.dt.float32,
}
_NP_DT_MAP = {
    "bf16": ml_dtypes.bfloat16,
    "f32r": np.float32,
    "f32": np.float32,
}


def build_moe_core_program(TC, D, H, V, mode="bf16", act="gelu"):
    """One NeuronCore's program: TC tokens, full V experts."""
    act_func = {
        "gelu": mybir.ActivationFunctionType.Gelu,
        "tanh": mybir.ActivationFunctionType.Tanh,  # sim-only (CoreSim lacks Gelu)
    }[act]
    DT = _DT_MAP[mode]
    f32 = mybir.dt.float32
    DC = D // P          # contraction chunks for MM1
    HC = H // P          # contraction chunks for MM2
    NT = min(512, TC)    # MM1 moving free dim (tokens)
    NK = min(512, H)     # MM2 moving free dim (output cols)
    TT = TC // P         # token tiles of 128

    # Bacc (not plain Bass): its finalize() runs generate_event_semaphores,
    # which splits multi-sem waits — TRN2 allows max 1 wait per instruction.
    nc = bacc.Bacc(trn_type="TRN2")
    x_t = nc.declare_dram_parameter("x_t", [D, TC], DT, isOutput=False)
    w1 = nc.declare_dram_parameter("w1", [V, D, H], DT, isOutput=False)
    w2 = nc.declare_dram_parameter("w2", [V, H, H], DT, isOutput=False)
    b1t = nc.declare_dram_parameter("b1t", [H, V], f32, isOutput=False)
    wbar = nc.declare_dram_parameter("wbar", [P, V], f32, isOutput=False)
    cbias = nc.declare_dram_parameter("cbias", [P, H], f32, isOutput=False)
    out = nc.declare_dram_parameter("out", [TC, H], f32, isOutput=True)

    with tile.TileContext(nc) as tc:
        with (
            tc.tile_pool(name="const", bufs=1) as cpool,
            tc.tile_pool(name="w1p", bufs=2) as w1p,
            tc.tile_pool(name="w2p", bufs=2) as w2p,
            tc.tile_pool(name="hbuf", bufs=1) as hpool,
            tc.tile_pool(name="accp", bufs=1) as accp,
            tc.tile_pool(name="ps1", bufs=4, space="PSUM") as ps1,
            tc.tile_pool(name="ps2", bufs=4, space="PSUM") as ps2,
        ):
            NTH = TC // NT  # token halves
            x_sb = cpool.tile([P, DC, TC], DT)
            nc.sync.dma_start(out=x_sb, in_=x_t.rearrange("(dc p) t -> p dc t", p=P))
            b1_sb = cpool.tile([P, HC, V], f32)
            nc.sync.dma_start(out=b1_sb, in_=b1t.rearrange("(hc p) v -> p hc v", p=P))
            wbar_sb = cpool.tile([P, V], f32)
            nc.sync.dma_start(out=wbar_sb, in_=wbar[:])
            cbias_sb = cpool.tile([P, H], f32)
            nc.sync.dma_start(out=cbias_sb, in_=cbias[:])
            # per-tt output accumulators: each tt's store depends only on its
            # own tile, so final DMAs overlap the last expert's compute.
            out_tiles = [
                accp.tile([P, H], f32, tag=f"acc{tt}", name=f"acc{tt}")
                for tt in range(TT)
            ]
            out_r = out.rearrange("(tt p) k -> p tt k", p=P)

            # Weight blocks are chained with explicit sync deps: each block's
            # descriptors only enqueue after the previous block's transfer
            # completes. The HW DGE engines service queues round-robin, so
            # without this the later blocks' packets steal bandwidth from the
            # startup-critical x + w1[0] transfers.
            prev_wdma = None

            def _chain(dma):
                nonlocal prev_wdma
                if prev_wdma is not None:
                    bass._add_dep_helper(
                        dma.ins, prev_wdma.ins, sync=True,
                        reason="serialize weight-block DMA issue",
                    )
                prev_wdma = dma

            for v in range(V):
                w1_sb = w1p.tile([P, DC, H], DT, tag="w1")
                _chain(nc.sync.dma_start(
                    out=w1_sb, in_=w1[v].rearrange("(dc p) h -> p dc h", p=P)
                ))
                h_sb = hpool.tile([P, HC, TC], DT, tag="h")

                # MM1 + gelu: produce h-major activations h_sb[h, t]
                for hc in range(HC):
                    for th in range(NTH):
                        p1 = ps1.tile([P, NT], f32, tag="p1")
                        for dc in range(DC):
                            nc.tensor.matmul(
                                p1,
                                w1_sb[:, dc, hc * P:(hc + 1) * P],
                                x_sb[:, dc, th * NT:(th + 1) * NT],
                                start=(dc == 0),
                                stop=(dc == DC - 1),
                            )
                        nc.scalar.activation(
                            h_sb[:, hc, th * NT:(th + 1) * NT],
                            p1,
                            act_func,
                            bias=b1_sb[:, hc, v:v + 1],
                        )

                # w2 emitted after MM1 + chained, so its transfer can't steal
                # bandwidth from the startup-critical x/w1[0] loads.
                w2_sb = w2p.tile([P, HC, H], DT, tag="w2")
                _chain(nc.sync.dma_start(
                    out=w2_sb, in_=w2[v].rearrange("(hc p) k -> p hc k", p=P)
                ))

                # MM2 + weighted accumulate into out_tiles[tt][t, k]
                for tt in range(TT):
                    for kc in range(H // NK):
                        p2 = ps2.tile([P, NK], f32, tag="p2")
                        for hc in range(HC):
                            nc.tensor.matmul(
                                p2,
                                h_sb[:, hc, tt * P:(tt + 1) * P],
                                w2_sb[:, hc, kc * NK:(kc + 1) * NK],
                                start=(hc == 0),
                                stop=(hc == HC - 1),
                            )
                        ksl = slice(kc * NK, (kc + 1) * NK)
                        in1 = cbias_sb[:, ksl] if v == 0 else out_tiles[tt][:, ksl]
                        nc.vector.scalar_tensor_tensor(
                            out=out_tiles[tt][:, ksl],
                            in0=p2,
                            scalar=wbar_sb[:, v:v + 1],
                            in1=in1,
                            op0=mybir.AluOpType.mult,
                            op1=mybir.AluOpType.add,
                        )
                    if v == V - 1:
                        # store this tt as soon as its accumulation finishes
                        nc.sync.dma_start(out=out_r[:, tt, :], in_=out_tiles[tt][:])
    nc.finalize()  # Bacc: runs wait-splitting + reg alloc passes
    return nc


_prog_cache = {}


def _get_program(mode, TC, D, H, V):
    key = (mode, TC, D, H, V)
    if key not in _prog_cache:
        _prog_cache[key] = build_moe_core_program(TC, D, H, V, mode=mode)
    return _prog_cache[key]


def host_prep(op_logits, token_feats, W1, b1, W2, b2, mode):
    """Shared host-side preprocessing: softmax folding, transpose, cast, shard."""
    op_logits = np.asarray(op_logits, dtype=np.float32)
    token_feats = np.asarray(token_feats, dtype=np.float32)
    W1 = np.asarray(W1, dtype=np.float32)
    b1 = np.asarray(b1, dtype=np.float32)
    W2 = np.asarray(W2, dtype=np.float32)
    b2 = np.asarray(b2, dtype=np.float32)

    B, T, D = token_feats.shape
    V, _, H = W1.shape
    BT = B * T
    TC = BT // N_CORES

    lg = op_logits.astype(np.float64)
    e = np.exp(lg - lg.max(axis=-1, keepdims=True))
    w = e / e.sum(axis=-1, keepdims=True)
    wbar = w.mean(axis=1)                       # [B, V], includes the 1/L
    cbias = wbar @ b2.astype(np.float64)        # [B, H]

    np_dt = _NP_DT_MAP[mode]
    x_t = np.ascontiguousarray(token_feats.reshape(BT, D).T).astype(np_dt)
    w1c = np.ascontiguousarray(W1.astype(np_dt))
    w2c = np.ascontiguousarray(W2.astype(np_dt))
    b1t = np.ascontiguousarray(b1.T.astype(np.float32))

    in_maps = []
    for c in range(N_CORES):
        bc = (c * TC) // T
        in_maps.append({
            "x_t": np.ascontiguousarray(x_t[:, c * TC:(c + 1) * TC]),
            "w1": w1c,
            "w2": w2c,
            "b1t": b1t,
            "wbar": np.ascontiguousarray(
                np.broadcast_to(wbar[bc].astype(np.float32), (P, V))
            ),
            "cbias": np.ascontiguousarray(
                np.broadcast_to(cbias[bc].astype(np.float32), (P, H))
            ),
        })
    return in_maps, (B, T, D, H, V, TC)


LAST_RESULTS = None


def kernel(op_logits, token_feats, W1, b1, W2, b2):
    global LAST_RESULTS
    mode = os.environ.get("MOE_DTYPE", "bf16")
    in_maps, (B, T, D, H, V, TC) = host_prep(
        op_logits, token_feats, W1, b1, W2, b2, mode
    )
    nc = _get_program(mode, TC, D, H, V)
    res = run_bass_kernel_spmd(
        nc,
        in_maps,
        list(range(N_CORES)),
        trace=os.environ.get("MOE_TRACE", "0") == "1",
    )
    LAST_RESULTS = res
    outs = [res.results[c]["out"] for c in range(N_CORES)]
    return np.concatenate(outs, axis=0).reshape(B, T, H).astype(np.float32)


# revision 16
# speedup vs baseline: 1.0222x; 1.0095x over previous
# MoE routing kernel for Trainium2 (Bass/Tile), SPMD over 8 NeuronCores.
#
# Reference computation (B=4, T=2048, D=H=1024, V=8, L=4):
#   h      = gelu(einsum("btd,vdh->btvh", X, W1) + b1)
#   outs   = einsum("btvh,vhk->btvk", h, W2) + b2
#   w      = softmax(op_logits, axis=-1)            # [B, L, V]
#   result = einsum("blv,btvh->bth", w, outs) / L
#
# Strategy:
#   - Host: softmax + mean over L -> wbar[B, V]; fold b2 into a single
#     per-batch combined bias cbias[b] = sum_v wbar[b,v] * b2[v].
#   - Data parallel over tokens: core c owns tokens [c*1024, (c+1)*1024).
#     Each 1024-token shard lies inside a single batch row b, so wbar/cbias
#     are per-core constants (shipped as data => one SPMD program).
#   - Per core, per expert v:
#       MM1:  pre1^T[h, t] = sum_d W1[v][d, h]^T-free X^T[d, t]   (PE)
#       gelu: h_sb[h, t] = Gelu(pre1 + b1[v][h])                  (ACT, PSUM->SBUF)
#       MM2:  out[t, k]  = sum_h h_sb[h, t]-as-lhsT W2[v][h, k]   (PE)
#       acc:  out_acc[t, k] = wbar[v] * out + (cbias | out_acc)   (DVE)
#   - X is pre-transposed on host to [D, BT] so every matmul operand is
#     naturally contraction-major; no on-device transposes anywhere.

import os

import numpy as np
import ml_dtypes

import concourse.bass as bass
import concourse.mybir as mybir
import concourse.tile as tile
from concourse import bacc
from concourse.bass_utils import run_bass_kernel_spmd

N_CORES = 8
P = 128

_DT_MAP = {
    "bf16": mybir.dt.bfloat16,
    "f32r": mybir.dt.float32r,
    "f32": mybir.dt.float32,
}
_NP_DT_MAP = {
    "bf16": ml_dtypes.bfloat16,
    "f32r": np.float32,
    "f32": np.float32,
}


def build_moe_core_program(TC, D, H, V, mode="bf16", act="gelu"):
    """One NeuronCore's program: TC tokens, full V experts."""
    act_func = {
        "gelu": mybir.ActivationFunctionType.Gelu,
        "tanh": mybir.ActivationFunctionType.Tanh,  # sim-only (CoreSim lacks Gelu)
    }[act]
    DT = _DT_MAP[mode]
    f32 = mybir.dt.float32
    DC = D // P          # contraction chunks for MM1
    HC = H // P          # contraction chunks for MM2
    NT = min(512, TC)    # MM1 moving free dim (tokens)
    NK = min(512, H)     # MM2 moving free dim (output cols)
    TT = TC // P         # token tiles of 128

    # Bacc (not plain Bass): its finalize() runs generate_event_semaphores,
    # which splits multi-sem waits — TRN2 allows max 1 wait per instruction.
    nc = bacc.Bacc(trn_type="TRN2")
    x_t = nc.declare_dram_parameter("x_t", [D, TC], DT, isOutput=False)
    w1 = nc.declare_dram_parameter("w1", [V, D, H], DT, isOutput=False)
    w2 = nc.declare_dram_parameter("w2", [V, H, H], DT, isOutput=False)
    b1t = nc.declare_dram_parameter("b1t", [H, V], f32, isOutput=False)
    wbar = nc.declare_dram_parameter("wbar", [P, V], f32, isOutput=False)
    cbias = nc.declare_dram_parameter("cbias", [P, H], f32, isOutput=False)
    out = nc.declare_dram_parameter("out", [TC, H], f32, isOutput=True)

    with tile.TileContext(nc) as tc:
        with (
            tc.tile_pool(name="const", bufs=1) as cpool,
            tc.tile_pool(name="w1p", bufs=2) as w1p,
            tc.tile_pool(name="w2p", bufs=2) as w2p,
            tc.tile_pool(name="hbuf", bufs=1) as hpool,
            tc.tile_pool(name="accp", bufs=1) as accp,
            tc.tile_pool(name="ps1", bufs=4, space="PSUM") as ps1,
            tc.tile_pool(name="ps2", bufs=4, space="PSUM") as ps2,
        ):
            NTH = TC // NT  # token halves
            # x per-dc tiles (2KB DMA lines): matmul deps are per-chunk, so PE
            # starts as soon as the first chunks land instead of waiting for
            # the whole 2MB transfer.
            x_tiles = []
            for dc in range(DC):
                xt = cpool.tile([P, TC], DT, tag=f"x{dc}", name=f"x{dc}")
                nc.sync.dma_start(out=xt, in_=x_t[dc * P:(dc + 1) * P, :])
                x_tiles.append(xt)
            b1_sb = cpool.tile([P, HC, V], f32)
            nc.sync.dma_start(out=b1_sb, in_=b1t.rearrange("(hc p) v -> p hc v", p=P))
            wbar_sb = cpool.tile([P, V], f32)
            nc.sync.dma_start(out=wbar_sb, in_=wbar[:])
            cbias_sb = cpool.tile([P, H], f32)
            nc.sync.dma_start(out=cbias_sb, in_=cbias[:])
            # per-tt output accumulators: each tt's store depends only on its
            # own tile, so final DMAs overlap the last expert's compute.
            out_tiles = [
                accp.tile([P, H], f32, tag=f"acc{tt}", name=f"acc{tt}")
                for tt in range(TT)
            ]
            out_r = out.rearrange("(tt p) k -> p tt k", p=P)

            # Weight blocks are chained with explicit sync deps: each block's
            # descriptors only enqueue after the previous block's transfer
            # completes. The HW DGE engines service queues round-robin, so
            # without this the later blocks' packets steal bandwidth from the
            # startup-critical x + w1[0] transfers. Expert 0's w1 chunks are
            # the chain head (issue immediately, alongside x).
            prev_block = None  # last DMA of the previous weight block

            def _dep_on_prev(dma):
                if prev_block is not None:
                    bass._add_dep_helper(
                        dma.ins, prev_block.ins, sync=True,
                        reason="serialize weight-block DMA issue",
                    )

            for v in range(V):
                # w1 per-dc tiles: first expert's matmuls begin after the
                # first ~256KB chunk instead of the full 2MB block.
                w1_tiles = []
                for dc in range(DC):
                    w1t = w1p.tile([P, H], DT, tag=f"w1_{dc}", name=f"w1_{dc}")
                    w1dma = nc.sync.dma_start(
                        out=w1t, in_=w1[v, dc * P:(dc + 1) * P, :]
                    )
                    _dep_on_prev(w1dma)
                    w1_tiles.append(w1t)
                prev_block = w1dma
                h_sb = hpool.tile([P, HC, TC], DT, tag="h")

                # MM1 + gelu: produce h-major activations h_sb[h, t]
                for hc in range(HC):
                    for th in range(NTH):
                        p1 = ps1.tile([P, NT], f32, tag="p1")
                        for dc in range(DC):
                            nc.tensor.matmul(
                                p1,
                                w1_tiles[dc][:, hc * P:(hc + 1) * P],
                                x_tiles[dc][:, th * NT:(th + 1) * NT],
                                start=(dc == 0),
                                stop=(dc == DC - 1),
                            )
                        nc.scalar.activation(
                            h_sb[:, hc, th * NT:(th + 1) * NT],
                            p1,
                            act_func,
                            bias=b1_sb[:, hc, v:v + 1],
                        )

                # w2 emitted after MM1 + chained, so its transfer can't steal
                # bandwidth from the startup-critical x/w1[0] loads.
                w2_sb = w2p.tile([P, HC, H], DT, tag="w2")
                w2dma = nc.sync.dma_start(
                    out=w2_sb, in_=w2[v].rearrange("(hc p) k -> p hc k", p=P)
                )
                _dep_on_prev(w2dma)
                prev_block = w2dma

                # MM2 + weighted accumulate into out_tiles[tt][t, k]
                for tt in range(TT):
                    for kc in range(H // NK):
                        p2 = ps2.tile([P, NK], f32, tag="p2")
                        for hc in range(HC):
                            nc.tensor.matmul(
                                p2,
                                h_sb[:, hc, tt * P:(tt + 1) * P],
                                w2_sb[:, hc, kc * NK:(kc + 1) * NK],
                                start=(hc == 0),
                                stop=(hc == HC - 1),
                            )
                        ksl = slice(kc * NK, (kc + 1) * NK)
                        in1 = cbias_sb[:, ksl] if v == 0 else out_tiles[tt][:, ksl]
                        nc.vector.scalar_tensor_tensor(
                            out=out_tiles[tt][:, ksl],
                            in0=p2,
                            scalar=wbar_sb[:, v:v + 1],
                            in1=in1,
                            op0=mybir.AluOpType.mult,
                            op1=mybir.AluOpType.add,
                        )
                    if v == V - 1:
                        # store this tt as soon as its accumulation finishes
                        nc.sync.dma_start(out=out_r[:, tt, :], in_=out_tiles[tt][:])
    nc.finalize()  # Bacc: runs wait-splitting + reg alloc passes
    return nc


_prog_cache = {}


def _get_program(mode, TC, D, H, V):
    key = (mode, TC, D, H, V)
    if key not in _prog_cache:
        _prog_cache[key] = build_moe_core_program(TC, D, H, V, mode=mode)
    return _prog_cache[key]


def host_prep(op_logits, token_feats, W1, b1, W2, b2, mode):
    """Shared host-side preprocessing: softmax folding, transpose, cast, shard."""
    op_logits = np.asarray(op_logits, dtype=np.float32)
    token_feats = np.asarray(token_feats, dtype=np.float32)
    W1 = np.asarray(W1, dtype=np.float32)
    b1 = np.asarray(b1, dtype=np.float32)
    W2 = np.asarray(W2, dtype=np.float32)
    b2 = np.asarray(b2, dtype=np.float32)

    B, T, D = token_feats.shape
    V, _, H = W1.shape
    BT = B * T
    TC = BT // N_CORES

    lg = op_logits.astype(np.float64)
    e = np.exp(lg - lg.max(axis=-1, keepdims=True))
    w = e / e.sum(axis=-1, keepdims=True)
    wbar = w.mean(axis=1)                       # [B, V], includes the 1/L
    cbias = wbar @ b2.astype(np.float64)        # [B, H]

    np_dt = _NP_DT_MAP[mode]
    x_t = np.ascontiguousarray(token_feats.reshape(BT, D).T).astype(np_dt)
    w1c = np.ascontiguousarray(W1.astype(np_dt))
    w2c = np.ascontiguousarray(W2.astype(np_dt))
    b1t = np.ascontiguousarray(b1.T.astype(np.float32))

    in_maps = []
    for c in range(N_CORES):
        bc = (c * TC) // T
        in_maps.append({
            "x_t": np.ascontiguousarray(x_t[:, c * TC:(c + 1) * TC]),
            "w1": w1c,
            "w2": w2c,
            "b1t": b1t,
            "wbar": np.ascontiguousarray(
                np.broadcast_to(wbar[bc].astype(np.float32), (P, V))
            ),
            "cbias": np.ascontiguousarray(
                np.broadcast_to(cbias[bc].astype(np.float32), (P, H))
            ),
        })
    return in_maps, (B, T, D, H, V, TC)


LAST_RESULTS = None


def kernel(op_logits, token_feats, W1, b1, W2, b2):
    global LAST_RESULTS
    mode = os.environ.get("MOE_DTYPE", "bf16")
    in_maps, (B, T, D, H, V, TC) = host_prep(
        op_logits, token_feats, W1, b1, W2, b2, mode
    )
    nc = _get_program(mode, TC, D, H, V)
    res = run_bass_kernel_spmd(
        nc,
        in_maps,
        list(range(N_CORES)),
        trace=os.environ.get("MOE_TRACE", "0") == "1",
    )
    LAST_RESULTS = res
    outs = [res.results[c]["out"] for c in range(N_CORES)]
    return np.concatenate(outs, axis=0).reshape(B, T, H).astype(np.float32)
